# revision 8
# baseline (speedup 1.0000x reference)
"""CFConv (SchNet continuous-filter convolution) kernel for Trainium2, 8 NeuronCores.

Computation (reference):
    f    = x @ W_in2fac                      # (NA, 128)
    f_j  = f[idx_j]                          # (NI, 128) gather
    wf   = w * f_j                           # elementwise
    conv = segment_sum(wf, seg_i, NA)        # (NA, 128), seg_i sorted
    y    = conv @ W_fac2out + b_fac2out      # (NA, 128)

Distribution strategy (graph partition by atom):
  * Atoms are sharded contiguously across the 8 cores (12500 atoms each).
    seg_i is sorted, so each core owns a contiguous interaction slice.
  * The gather f[idx_j] is eliminated entirely: idx_j is known on the host,
    so the host pre-gathers the RAW x rows into the per-core interaction
    stream (pure data movement), and each core computes f_j = x_j @ W_in2fac
    with one matmul per 128-interaction tile.  This removes the SWDGE
    descriptor-rate bottleneck and the f-table build of the old design.

On-core algorithm (per 128-interaction tile):
  * MM1: f_tile[slot, filt] = xgT_tile^T @ W1   (lhsT = host-transposed x_j)
  * ACT: copy f PSUM fp32 -> SBUF fp16 (chunked [128,1024])
  * DVE: wf = ww * f (fp16 2x mode)
  * MM2: convT[filt, atoms] += wf^T @ S        (PSUM accumulation per block)
  * Regular/overflow layout: every atom owns exactly 16 slots -> for 90% of
    edges S is ONE constant [128, 8] matrix (slot p -> atom p//16) and MM2 is
    an N=8 matmul into the block's 8-column region.  Edges beyond the 16th
    per atom go to per-block overflow tiles whose S is built with the
    broadcast is_equal trick (only ~10% of the stream).
  * fac2out per 512-atom group (4 blocks / one PSUM bank): DVE copies convT
    to SBUF fp16, then yT = W2^T @ convT (W2 stationary, N=512) + bias outer
    ones; yT is stored transposed so the output DMA is contiguous per
    partition.  Host transposes yT back.
"""

import math
import sys

import numpy as np

import concourse.bass as bass
import concourse.mybir as mybir
import concourse.tile as tile
from concourse import bacc
from concourse.bass_utils import run_bass_kernel_spmd

F32 = mybir.dt.float32
F16 = mybir.dt.float16

NA = 100_000
NI = 1_600_000
N_CORES = 8
D = 128


class Cfg:
    def __init__(self, na=NA, ni=NI, n_cores=N_CORES, slots=16, chunk=1024,
                 slab=4096):
        self.na = na
        self.ni = ni
        self.n_cores = n_cores
        self.apc = na // n_cores            # atoms per core
        self.slots = slots                  # regular slots per atom
        self.apt = 128 // slots             # atoms per regular tile (8)
        self.nb = math.ceil(self.apc / 128)  # 128-atom blocks per core
        self.chunk = chunk                  # interactions per f/mul chunk
        self.slab = slab                    # interactions per DMA slab
        assert na % n_cores == 0
        assert 128 % slots == 0
        assert chunk % 128 == 0 and slab % chunk == 0


def _plan(seg, cfg):
    """Tile layout plan shared by all cores (tile counts maxed over cores)."""
    nb, apc, K = cfg.nb, cfg.apc, cfg.slots
    bounds = np.searchsorted(seg, np.arange(cfg.n_cores + 1) * apc)
    per_core = []
    ovf_cnt = np.zeros((cfg.n_cores, nb), dtype=np.int64)
    for c in range(cfg.n_cores):
        e0, e1 = int(bounds[c]), int(bounds[c + 1])
        ls = (seg[e0:e1] - c * apc).astype(np.int64)
        n = e1 - e0
        starts = np.searchsorted(ls, np.arange(apc + 1))
        occ = np.arange(n) - starts[ls]
        blk = ls >> 7
        q = ls & 127
        reg = occ < K
        ovf_cnt[c] = np.bincount(blk[~reg], minlength=nb)
        per_core.append(dict(e0=e0, e1=e1, ls=ls, occ=occ, blk=blk, q=q,
                             reg=reg))

    T_ov = np.ceil(ovf_cnt.max(axis=0) / 128.0).astype(np.int64)
    atoms_pb = np.full(nb, 128, dtype=np.int64)
    atoms_pb[-1] = apc - 128 * (nb - 1)
    R = np.ceil(atoms_pb * K / 128.0).astype(np.int64)
    tiles_pb = R + T_ov
    tile_base = np.concatenate([[0], np.cumsum(tiles_pb)])
    ov_base = np.concatenate([[0], np.cumsum(T_ov)])
    ntiles = int(tile_base[-1])
    n_ov = int(ov_base[-1])
    return dict(T_ov=T_ov, R=R, tile_base=tile_base, ov_base=ov_base,
                ntiles=ntiles, n_ov=n_ov, per_core=per_core)


def _pack_core(cfg, plan, c, x16, w, idx_j, w1_16, w2_16, bias_16):
    """Per-core host-side packing: positions + reordered fp16 streams."""
    K = cfg.slots
    pc = plan["per_core"][c]
    tile_base, R, ov_base = plan["tile_base"], plan["R"], plan["ov_base"]
    ntiles, n_ov = plan["ntiles"], plan["n_ov"]
    e0, e1 = pc["e0"], pc["e1"]
    ls, occ, blk, q, reg = pc["ls"], pc["occ"], pc["blk"], pc["q"], pc["reg"]
    n = e1 - e0

    pos = np.empty(n, dtype=np.int64)
    rb, rq, rocc = blk[reg], q[reg], occ[reg]
    pos[reg] = (tile_base[rb] + (rq >> 3)) * 128 + (rq & 7) * K + rocc

    ovf_es = np.flatnonzero(~reg)
    ob = blk[ovf_es]
    obs = np.searchsorted(ob, np.arange(cfg.nb + 1))
    oidx = np.arange(len(ovf_es)) - obs[ob]
    pos[ovf_es] = (tile_base[ob] + R[ob] + (oidx >> 7)) * 128 + (oidx & 127)

    sc = np.zeros(max(n_ov, 1) * 128, dtype=np.float16)
    ovtile = ov_base[ob] + (oidx >> 7)
    sc[ovtile * 128 + (oidx & 127)] = q[ovf_es].astype(np.float16)
    segov = np.ascontiguousarray(sc.reshape(max(n_ov, 1), 128).T)

    E = ntiles * 128
    wp16 = np.zeros((E, D), dtype=np.float16)
    wp16[pos] = w[e0:e1].astype(np.float16)
    ww = np.ascontiguousarray(
        wp16.reshape(ntiles, 128, D).transpose(1, 0, 2).reshape(128, E))

    xg = np.zeros((E, D), dtype=np.float16)
    xg[pos] = x16[idx_j[e0:e1]]
    xgT = np.ascontiguousarray(
        xg.reshape(ntiles, 128, D).transpose(2, 0, 1).reshape(128, E))

    s8 = np.zeros((128, cfg.apt), dtype=np.float16)
    s8[np.arange(128), np.arange(128) // K] = 1.0
    iota = np.tile(np.arange(128, dtype=np.float16), (128, 1))

    return {"xgT": xgT, "ww": ww, "segov": segov, "w1": w1_16, "w2": w2_16,
            "bias": bias_16, "s8": np.ascontiguousarray(s8),
            "iota": np.ascontiguousarray(iota)}


def _build(cfg, plan, skew=3, act_frac=5):
    """Build + compile the SPMD Bass program (identical for all cores).

    skew: number of chunks between MM1 emission and MM2 emission (software
    pipeline depth so the PE never waits on the ACT/DVE f-drain chain).
    act_frac: of every act_frac chunks, act_frac-1 drain f via ACT copy +
    DVE 2x multiply; 1 drains via DVE direct-from-PSUM multiply (balance).
    """
    from collections import deque
    from contextlib import ExitStack

    nb, K, apt = cfg.nb, cfg.slots, cfg.apt
    T_ov, R, tile_base, ov_base = (plan["T_ov"], plan["R"],
                                   plan["tile_base"], plan["ov_base"])
    ntiles, n_ov = plan["ntiles"], plan["n_ov"]
    E = ntiles * 128

    # per-tile meta: (block, kind, j_or_r, ov_id)
    meta = []
    for b in range(nb):
        for j in range(int(R[b])):
            meta.append((b, 0, j, -1))
        for r in range(int(T_ov[b])):
            meta.append((b, 1, r, int(ov_base[b]) + r))
    assert len(meta) == ntiles

    ngroups = math.ceil(nb / 4)
    grp_first = [int(tile_base[min(4 * g, nb)]) for g in range(ngroups)]
    grp_last = [int(tile_base[min(4 * g + 4, nb)]) - 1 for g in range(ngroups)]

    nc = bacc.Bacc("TRN2", target_bir_lowering=False, debug=False,
                   num_devices=cfg.n_cores)

    xgT_d = nc.dram_tensor("xgT", [128, E], F16, kind="ExternalInput")
    ww_d = nc.dram_tensor("ww", [128, E], F16, kind="ExternalInput")
    segov_d = nc.dram_tensor("segov", [128, max(n_ov, 1)], F16,
                             kind="ExternalInput")
    w1_d = nc.dram_tensor("w1", [D, D], F16, kind="ExternalInput")
    w2_d = nc.dram_tensor("w2", [D, D], F16, kind="ExternalInput")
    bias_d = nc.dram_tensor("bias", [1, D], F16, kind="ExternalInput")
    s8_d = nc.dram_tensor("s8", [128, apt], F16, kind="ExternalInput")
    iota_d = nc.dram_tensor("iota", [128, 128], F16, kind="ExternalInput")
    yT_d = nc.dram_tensor("yT", [D, cfg.apc], F16, kind="ExternalOutput")

    with tile.TileContext(nc) as tc, ExitStack() as ctx:
        cpool = ctx.enter_context(tc.tile_pool(name="const", bufs=1))
        xp = ctx.enter_context(tc.tile_pool(name="xgt", bufs=4))
        wp = ctx.enter_context(tc.tile_pool(name="wwt", bufs=4))
        fps = ctx.enter_context(tc.tile_pool(name="fps", bufs=2, space="PSUM"))
        fbp = ctx.enter_context(tc.tile_pool(name="fsb", bufs=5))
        wfp = ctx.enter_context(tc.tile_pool(name="wft", bufs=5))
        sp = ctx.enter_context(tc.tile_pool(name="sov", bufs=3))
        sps = ctx.enter_context(tc.tile_pool(name="conv", bufs=2,
                                             space="PSUM"))
        yps = ctx.enter_context(tc.tile_pool(name="yps", bufs=1,
                                             space="PSUM"))
        cvp = ctx.enter_context(tc.tile_pool(name="convsb", bufs=2))
        ybp = ctx.enter_context(tc.tile_pool(name="ysb", bufs=2))

        # ---- constants ----
        w1_t = cpool.tile([D, D], F16)
        nc.scalar.dma_start(out=w1_t[:], in_=w1_d[:, :])
        w2_t = cpool.tile([D, D], F16)
        nc.scalar.dma_start(out=w2_t[:], in_=w2_d[:, :])
        bias_t = cpool.tile([1, D], F16)
        nc.scalar.dma_start(out=bias_t[:], in_=bias_d[:, :])
        s8_t = cpool.tile([128, apt], F16)
        nc.scalar.dma_start(out=s8_t[:], in_=s8_d[:, :])
        iota_t = cpool.tile([128, 128], F16)
        nc.scalar.dma_start(out=iota_t[:], in_=iota_d[:, :])
        ones_t = cpool.tile([1, 512], F16)
        nc.vector.memset(ones_t[:], 1.0)
        if n_ov > 0:
            segov_t = cpool.tile([128, n_ov], F16)
            nc.scalar.dma_start(out=segov_t[:], in_=segov_d[:, :n_ov])

        grp_state = {}

        def finalize_group(g):
            conv_ps = grp_state.pop(g)
            convsb = cvp.tile([128, 512], F16)
            nc.vector.tensor_copy(convsb[:], conv_ps[:])
            ytp = yps.tile([128, 512], F32)
            nc.tensor.matmul(out=ytp[:], lhsT=w2_t[:], rhs=convsb[:],
                             start=True, stop=False)
            nc.tensor.matmul(out=ytp[:], lhsT=bias_t[:], rhs=ones_t[:],
                             start=False, stop=True)
            ysb = ybp.tile([128, 512], F16)
            nc.scalar.copy(ysb[:], ytp[:])
            a0 = g * 512
            m = min(512, cfg.apc - a0)
            nc.sync.dma_start(out=yT_d[:, a0:a0 + m], in_=ysb[:, :m])

        def emit_mm2s(pc):
            """Emit segment-sum matmuls for a previously prepared chunk."""
            t0, nt, wft, S, ov_off = pc
            for i in range(nt):
                t = t0 + i
                b, kind, j, ov = meta[t]
                g = b // 4
                if g not in grp_state:
                    grp_state[g] = sps.tile([128, 512], F32,
                                            name="conv_ps", tag="conv_ps")
                conv_ps = grp_state[g]
                first = (t == grp_first[g])
                last = (t == grp_last[g])
                if kind == 0:
                    col0 = (b % 4) * 128 + j * apt
                    rhs = s8_t[:, :apt]
                    ncols = apt
                else:
                    col0 = (b % 4) * 128
                    ii = ov_off[i]
                    rhs = S[:, ii * 128:(ii + 1) * 128]
                    ncols = 128
                nc.tensor.matmul(
                    out=conv_ps[:, col0:col0 + ncols],
                    lhsT=wft[:, i * 128:(i + 1) * 128],
                    rhs=rhs, start=first, stop=last)
                if last:
                    finalize_group(g)

        # fixed overflow-S tile size: max ovf tiles in any slab
        kmax = 1
        for s0 in range(0, E, cfg.slab):
            st0, st1 = s0 // 128, min((s0 + cfg.slab) // 128, ntiles)
            kmax = max(kmax, sum(1 for t in range(st0, st1)
                                 if meta[t][1] == 1))

        pend = deque()
        cidx = 0
        for s0 in range(0, E, cfg.slab):
            sL = min(cfg.slab, E - s0)
            xgt = xp.tile([128, cfg.slab], F16)
            nc.sync.dma_start(out=xgt[:, :sL], in_=xgT_d[:, s0:s0 + sL])
            wwt = wp.tile([128, cfg.slab], F16)
            nc.scalar.dma_start(out=wwt[:, :sL], in_=ww_d[:, s0:s0 + sL])

            # overflow S matrices for the whole slab in one is_equal
            st0, st1 = s0 // 128, (s0 + sL) // 128
            ov_tiles = [t for t in range(st0, st1) if meta[t][1] == 1]
            S = None
            ov_off = {}
            if ov_tiles:
                k = len(ov_tiles)
                o0 = meta[ov_tiles[0]][3]
                ov_off = {t: ii for ii, t in enumerate(ov_tiles)}
                S = sp.tile([128, 128 * kmax], F16, name="sov", tag="sov")
                io = iota_t[:, :128]
                iob = bass.AP(io.tensor, io.offset,
                              [list(io.ap[0]), [0, k], list(io.ap[1])])
                nc.vector.tensor_tensor(
                    out=S[:, :k * 128].rearrange("p (t q) -> p t q", q=128),
                    in0=segov_t[:, o0:o0 + k].to_broadcast([128, k, 128]),
                    in1=iob, op=mybir.AluOpType.is_equal)

            for c0 in range(0, sL, cfg.chunk):
                cL = min(cfg.chunk, sL - c0)
                nt = cL // 128
                t0 = (s0 + c0) // 128

                psf = fps.tile([128, cfg.chunk], F32)
                for i in range(nt):
                    nc.tensor.matmul(
                        out=psf[:, i * 128:(i + 1) * 128],
                        lhsT=xgt[:, c0 + i * 128:c0 + (i + 1) * 128],
                        rhs=w1_t[:], start=True, stop=True)
                wft = wfp.tile([128, cfg.chunk], F16)
                if cidx % act_frac != act_frac - 1:
                    fsb = fbp.tile([128, cfg.chunk], F16)
                    nc.scalar.copy(fsb[:, :cL], psf[:, :cL])
                    nc.vector.tensor_mul(wft[:, :cL], wwt[:, c0:c0 + cL],
                                         fsb[:, :cL])
                else:
                    nc.vector.tensor_mul(wft[:, :cL], wwt[:, c0:c0 + cL],
                                         psf[:, :cL])
                cidx += 1

                ov_local = {i: ov_off[t0 + i] for i in range(nt)
                            if (t0 + i) in ov_off}
                pend.append((t0, nt, wft, S, ov_local))
                if len(pend) > skew:
                    emit_mm2s(pend.popleft())

        while pend:
            emit_mm2s(pend.popleft())
        assert not grp_state

    nc.compile()
    return nc


def _run(inputs, cfg=None, trace=False, tmpdir=None):
    x = np.asarray(inputs["x"], dtype=np.float32)
    w = np.asarray(inputs["w"], dtype=np.float32)
    seg = np.asarray(inputs["seg_i"]).astype(np.int64)
    idx_j = np.asarray(inputs["idx_j"]).astype(np.int64)
    W1 = np.asarray(inputs["W_in2fac"], dtype=np.float32)
    W2 = np.asarray(inputs["W_fac2out"], dtype=np.float32)
    b = np.asarray(inputs["b_fac2out"], dtype=np.float32)

    if cfg is None:
        cfg = Cfg()

    plan = _plan(seg, cfg)

    x16 = x.astype(np.float16)
    w1_16 = np.ascontiguousarray(W1.astype(np.float16))
    w2_16 = np.ascontiguousarray(W2.astype(np.float16))
    bias_16 = np.ascontiguousarray(b[None, :].astype(np.float16))

    in_maps = []
    for c in range(cfg.n_cores):
        in_maps.append(_pack_core(cfg, plan, c, x16, w, idx_j,
                                  w1_16, w2_16, bias_16))

    nc = _build(cfg, plan)

    res = run_bass_kernel_spmd(nc, in_maps, core_ids=list(range(cfg.n_cores)),
                               tmpdir=tmpdir, trace=trace)
    y = np.concatenate(
        [np.asarray(res.results[c]["yT"]).astype(np.float32).T
         for c in range(cfg.n_cores)], axis=0)
    return y[:cfg.na], res, nc, in_maps


def kernel(**inputs) -> np.ndarray:
    y, _res, _nc, _maps = _run(inputs)
    return y


# revision 9
# speedup vs baseline: 1.1569x; 1.1569x over previous
"""CFConv (SchNet continuous-filter convolution) kernel for Trainium2, 8 NeuronCores.

Computation (reference):
    f    = x @ W_in2fac                      # (NA, 128)
    f_j  = f[idx_j]                          # (NI, 128) gather
    wf   = w * f_j                           # elementwise
    conv = segment_sum(wf, seg_i, NA)        # (NA, 128), seg_i sorted
    y    = conv @ W_fac2out + b_fac2out      # (NA, 128)

Distribution strategy (graph partition by atom):
  * Atoms are sharded contiguously across the 8 cores (12500 atoms each).
    seg_i is sorted, so each core owns a contiguous interaction slice.
  * The gather f[idx_j] is eliminated entirely: idx_j is known on the host,
    so the host pre-gathers the RAW x rows into the per-core interaction
    stream (pure data movement), and each core computes f_j = x_j @ W_in2fac
    with one matmul per 128-interaction tile.  This removes the SWDGE
    descriptor-rate bottleneck and the f-table build of the old design.

On-core algorithm (per 128-interaction tile):
  * MM1: f_tile[slot, filt] = xgT_tile^T @ W1   (lhsT = host-transposed x_j)
  * ACT: copy f PSUM fp32 -> SBUF fp16 (chunked [128,1024])
  * DVE: wf = ww * f (fp16 2x mode)
  * MM2: convT[filt, atoms] += wf^T @ S        (PSUM accumulation per block)
  * Regular/overflow layout: every atom owns exactly 16 slots -> for 90% of
    edges S is ONE constant [128, 8] matrix (slot p -> atom p//16) and MM2 is
    an N=8 matmul into the block's 8-column region.  Edges beyond the 16th
    per atom go to per-block overflow tiles whose S is built with the
    broadcast is_equal trick (only ~10% of the stream).
  * fac2out per 512-atom group (4 blocks / one PSUM bank): DVE copies convT
    to SBUF fp16, then yT = W2^T @ convT (W2 stationary, N=512) + bias outer
    ones; yT is stored transposed so the output DMA is contiguous per
    partition.  Host transposes yT back.
"""

import math
import sys

import numpy as np

import concourse.bass as bass
import concourse.mybir as mybir
import concourse.tile as tile
from concourse import bacc
from concourse.bass_utils import run_bass_kernel_spmd

F32 = mybir.dt.float32
F16 = mybir.dt.float16

NA = 100_000
NI = 1_600_000
N_CORES = 8
D = 128


class Cfg:
    def __init__(self, na=NA, ni=NI, n_cores=N_CORES, slots=16, chunk=1024,
                 slab=8192):
        self.na = na
        self.ni = ni
        self.n_cores = n_cores
        self.apc = na // n_cores            # atoms per core
        self.slots = slots                  # regular slots per atom
        self.apt = 128 // slots             # atoms per regular tile (8)
        self.nb = math.ceil(self.apc / 128)  # 128-atom blocks per core
        self.chunk = chunk                  # interactions per f/mul chunk
        self.slab = slab                    # interactions per DMA slab
        assert na % n_cores == 0
        assert 128 % slots == 0
        assert chunk % 128 == 0 and slab % chunk == 0


def _plan(seg, cfg):
    """Tile layout plan shared by all cores (tile counts maxed over cores)."""
    nb, apc, K = cfg.nb, cfg.apc, cfg.slots
    bounds = np.searchsorted(seg, np.arange(cfg.n_cores + 1) * apc)
    per_core = []
    ovf_cnt = np.zeros((cfg.n_cores, nb), dtype=np.int64)
    for c in range(cfg.n_cores):
        e0, e1 = int(bounds[c]), int(bounds[c + 1])
        ls = (seg[e0:e1] - c * apc).astype(np.int64)
        n = e1 - e0
        starts = np.searchsorted(ls, np.arange(apc + 1))
        occ = np.arange(n) - starts[ls]
        blk = ls >> 7
        q = ls & 127
        reg = occ < K
        ovf_cnt[c] = np.bincount(blk[~reg], minlength=nb)
        per_core.append(dict(e0=e0, e1=e1, ls=ls, occ=occ, blk=blk, q=q,
                             reg=reg))

    T_ov = np.ceil(ovf_cnt.max(axis=0) / 128.0).astype(np.int64)
    atoms_pb = np.full(nb, 128, dtype=np.int64)
    atoms_pb[-1] = apc - 128 * (nb - 1)
    R = np.ceil(atoms_pb * K / 128.0).astype(np.int64)
    tiles_pb = R + T_ov
    tile_base = np.concatenate([[0], np.cumsum(tiles_pb)])
    ov_base = np.concatenate([[0], np.cumsum(T_ov)])
    ntiles = int(tile_base[-1])
    n_ov = int(ov_base[-1])
    return dict(T_ov=T_ov, R=R, tile_base=tile_base, ov_base=ov_base,
                ntiles=ntiles, n_ov=n_ov, per_core=per_core)


def _pack_core(cfg, plan, c, x16, w, idx_j, w1_16, w2_16, bias_16):
    """Per-core host-side packing: positions + reordered fp16 streams."""
    K = cfg.slots
    pc = plan["per_core"][c]
    tile_base, R, ov_base = plan["tile_base"], plan["R"], plan["ov_base"]
    ntiles, n_ov = plan["ntiles"], plan["n_ov"]
    e0, e1 = pc["e0"], pc["e1"]
    ls, occ, blk, q, reg = pc["ls"], pc["occ"], pc["blk"], pc["q"], pc["reg"]
    n = e1 - e0

    pos = np.empty(n, dtype=np.int64)
    rb, rq, rocc = blk[reg], q[reg], occ[reg]
    pos[reg] = (tile_base[rb] + (rq >> 3)) * 128 + (rq & 7) * K + rocc

    ovf_es = np.flatnonzero(~reg)
    ob = blk[ovf_es]
    obs = np.searchsorted(ob, np.arange(cfg.nb + 1))
    oidx = np.arange(len(ovf_es)) - obs[ob]
    pos[ovf_es] = (tile_base[ob] + R[ob] + (oidx >> 7)) * 128 + (oidx & 127)

    sc = np.zeros(max(n_ov, 1) * 128, dtype=np.float16)
    ovtile = ov_base[ob] + (oidx >> 7)
    sc[ovtile * 128 + (oidx & 127)] = q[ovf_es].astype(np.float16)
    segov = np.ascontiguousarray(sc.reshape(max(n_ov, 1), 128).T)

    E = ntiles * 128
    wp16 = np.zeros((E, D), dtype=np.float16)
    wp16[pos] = w[e0:e1].astype(np.float16)
    ww = np.ascontiguousarray(
        wp16.reshape(ntiles, 128, D).transpose(1, 0, 2).reshape(128, E))

    xg = np.zeros((E, D), dtype=np.float16)
    xg[pos] = x16[idx_j[e0:e1]]
    xgT = np.ascontiguousarray(
        xg.reshape(ntiles, 128, D).transpose(2, 0, 1).reshape(128, E))

    s8 = np.zeros((128, cfg.apt), dtype=np.float16)
    s8[np.arange(128), np.arange(128) // K] = 1.0
    iota = np.tile(np.arange(128, dtype=np.float16), (128, 1))

    return {"xgT": xgT, "ww": ww, "segov": segov, "w1": w1_16, "w2": w2_16,
            "bias": bias_16, "s8": np.ascontiguousarray(s8),
            "iota": np.ascontiguousarray(iota)}


def _build(cfg, plan, skew=3, act_frac=5):
    """Build + compile the SPMD Bass program (identical for all cores).

    skew: number of chunks between MM1 emission and MM2 emission (software
    pipeline depth so the PE never waits on the ACT/DVE f-drain chain).
    act_frac: of every act_frac chunks, act_frac-1 drain f via ACT copy +
    DVE 2x multiply; 1 drains via DVE direct-from-PSUM multiply (balance).
    """
    from collections import deque
    from contextlib import ExitStack

    nb, K, apt = cfg.nb, cfg.slots, cfg.apt
    T_ov, R, tile_base, ov_base = (plan["T_ov"], plan["R"],
                                   plan["tile_base"], plan["ov_base"])
    ntiles, n_ov = plan["ntiles"], plan["n_ov"]
    E = ntiles * 128

    # per-tile meta: (block, kind, j_or_r, ov_id)
    meta = []
    for b in range(nb):
        for j in range(int(R[b])):
            meta.append((b, 0, j, -1))
        for r in range(int(T_ov[b])):
            meta.append((b, 1, r, int(ov_base[b]) + r))
    assert len(meta) == ntiles

    ngroups = math.ceil(nb / 4)
    grp_first = [int(tile_base[min(4 * g, nb)]) for g in range(ngroups)]
    grp_last = [int(tile_base[min(4 * g + 4, nb)]) - 1 for g in range(ngroups)]

    nc = bacc.Bacc("TRN2", target_bir_lowering=False, debug=False,
                   num_devices=cfg.n_cores)

    xgT_d = nc.dram_tensor("xgT", [128, E], F16, kind="ExternalInput")
    ww_d = nc.dram_tensor("ww", [128, E], F16, kind="ExternalInput")
    segov_d = nc.dram_tensor("segov", [128, max(n_ov, 1)], F16,
                             kind="ExternalInput")
    w1_d = nc.dram_tensor("w1", [D, D], F16, kind="ExternalInput")
    w2_d = nc.dram_tensor("w2", [D, D], F16, kind="ExternalInput")
    bias_d = nc.dram_tensor("bias", [1, D], F16, kind="ExternalInput")
    s8_d = nc.dram_tensor("s8", [128, apt], F16, kind="ExternalInput")
    iota_d = nc.dram_tensor("iota", [128, 128], F16, kind="ExternalInput")
    yT_d = nc.dram_tensor("yT", [D, cfg.apc], F16, kind="ExternalOutput")

    with tile.TileContext(nc) as tc, ExitStack() as ctx:
        cpool = ctx.enter_context(tc.tile_pool(name="const", bufs=1))
        xp = ctx.enter_context(tc.tile_pool(name="xgt", bufs=3))
        wp = ctx.enter_context(tc.tile_pool(name="wwt", bufs=3))
        fps = ctx.enter_context(tc.tile_pool(name="fps", bufs=2, space="PSUM"))
        fbp = ctx.enter_context(tc.tile_pool(name="fsb", bufs=5))
        wfp = ctx.enter_context(tc.tile_pool(name="wft", bufs=5))
        sp = ctx.enter_context(tc.tile_pool(name="sov", bufs=3))
        sps = ctx.enter_context(tc.tile_pool(name="conv", bufs=2,
                                             space="PSUM"))
        yps = ctx.enter_context(tc.tile_pool(name="yps", bufs=1,
                                             space="PSUM"))
        cvp = ctx.enter_context(tc.tile_pool(name="convsb", bufs=2))
        ybp = ctx.enter_context(tc.tile_pool(name="ysb", bufs=2))

        # ---- constants ----
        w1_t = cpool.tile([D, D], F16)
        nc.scalar.dma_start(out=w1_t[:], in_=w1_d[:, :])
        w2_t = cpool.tile([D, D], F16)
        nc.scalar.dma_start(out=w2_t[:], in_=w2_d[:, :])
        bias_t = cpool.tile([1, D], F16)
        nc.scalar.dma_start(out=bias_t[:], in_=bias_d[:, :])
        s8_t = cpool.tile([128, apt], F16)
        nc.scalar.dma_start(out=s8_t[:], in_=s8_d[:, :])
        iota_t = cpool.tile([128, 128], F16)
        nc.scalar.dma_start(out=iota_t[:], in_=iota_d[:, :])
        ones_t = cpool.tile([1, 512], F16)
        nc.vector.memset(ones_t[:], 1.0)
        if n_ov > 0:
            segov_t = cpool.tile([128, n_ov], F16)
            nc.scalar.dma_start(out=segov_t[:], in_=segov_d[:, :n_ov])

        grp_state = {}

        def finalize_group(g):
            conv_ps = grp_state.pop(g)
            convsb = cvp.tile([128, 512], F16)
            nc.vector.tensor_copy(convsb[:], conv_ps[:])
            ytp = yps.tile([128, 512], F32)
            nc.tensor.matmul(out=ytp[:], lhsT=w2_t[:], rhs=convsb[:],
                             start=True, stop=False)
            nc.tensor.matmul(out=ytp[:], lhsT=bias_t[:], rhs=ones_t[:],
                             start=False, stop=True)
            ysb = ybp.tile([128, 512], F16)
            nc.scalar.copy(ysb[:], ytp[:])
            a0 = g * 512
            m = min(512, cfg.apc - a0)
            nc.sync.dma_start(out=yT_d[:, a0:a0 + m], in_=ysb[:, :m])

        def emit_mm2s(pc):
            """Emit segment-sum matmuls for a previously prepared chunk."""
            t0, nt, wft, S, ov_off = pc
            for i in range(nt):
                t = t0 + i
                b, kind, j, ov = meta[t]
                g = b // 4
                if g not in grp_state:
                    grp_state[g] = sps.tile([128, 512], F32,
                                            name="conv_ps", tag="conv_ps")
                conv_ps = grp_state[g]
                first = (t == grp_first[g])
                last = (t == grp_last[g])
                if kind == 0:
                    col0 = (b % 4) * 128 + j * apt
                    rhs = s8_t[:, :apt]
                    ncols = apt
                else:
                    col0 = (b % 4) * 128
                    ii = ov_off[i]
                    rhs = S[:, ii * 128:(ii + 1) * 128]
                    ncols = 128
                nc.tensor.matmul(
                    out=conv_ps[:, col0:col0 + ncols],
                    lhsT=wft[:, i * 128:(i + 1) * 128],
                    rhs=rhs, start=first, stop=last)
                if last:
                    finalize_group(g)

        # fixed overflow-S tile size: max ovf tiles in any slab
        kmax = 1
        for s0 in range(0, E, cfg.slab):
            st0, st1 = s0 // 128, min((s0 + cfg.slab) // 128, ntiles)
            kmax = max(kmax, sum(1 for t in range(st0, st1)
                                 if meta[t][1] == 1))

        pend = deque()
        cidx = 0
        for s0 in range(0, E, cfg.slab):
            sL = min(cfg.slab, E - s0)
            xgt = xp.tile([128, cfg.slab], F16)
            nc.sync.dma_start(out=xgt[:, :sL], in_=xgT_d[:, s0:s0 + sL])
            wwt = wp.tile([128, cfg.slab], F16)
            nc.scalar.dma_start(out=wwt[:, :sL], in_=ww_d[:, s0:s0 + sL])

            # overflow S matrices for the whole slab in one is_equal
            st0, st1 = s0 // 128, (s0 + sL) // 128
            ov_tiles = [t for t in range(st0, st1) if meta[t][1] == 1]
            S = None
            ov_off = {}
            if ov_tiles:
                k = len(ov_tiles)
                o0 = meta[ov_tiles[0]][3]
                ov_off = {t: ii for ii, t in enumerate(ov_tiles)}
                S = sp.tile([128, 128 * kmax], F16, name="sov", tag="sov")
                io = iota_t[:, :128]
                iob = bass.AP(io.tensor, io.offset,
                              [list(io.ap[0]), [0, k], list(io.ap[1])])
                nc.vector.tensor_tensor(
                    out=S[:, :k * 128].rearrange("p (t q) -> p t q", q=128),
                    in0=segov_t[:, o0:o0 + k].to_broadcast([128, k, 128]),
                    in1=iob, op=mybir.AluOpType.is_equal)

            for c0 in range(0, sL, cfg.chunk):
                cL = min(cfg.chunk, sL - c0)
                nt = cL // 128
                t0 = (s0 + c0) // 128

                psf = fps.tile([128, cfg.chunk], F32)
                for i in range(nt):
                    nc.tensor.matmul(
                        out=psf[:, i * 128:(i + 1) * 128],
                        lhsT=xgt[:, c0 + i * 128:c0 + (i + 1) * 128],
                        rhs=w1_t[:], start=True, stop=True)
                wft = wfp.tile([128, cfg.chunk], F16)
                if cidx % act_frac != act_frac - 1:
                    fsb = fbp.tile([128, cfg.chunk], F16)
                    nc.scalar.copy(fsb[:, :cL], psf[:, :cL])
                    nc.vector.tensor_mul(wft[:, :cL], wwt[:, c0:c0 + cL],
                                         fsb[:, :cL])
                else:
                    nc.vector.tensor_mul(wft[:, :cL], wwt[:, c0:c0 + cL],
                                         psf[:, :cL])
                cidx += 1

                ov_local = {i: ov_off[t0 + i] for i in range(nt)
                            if (t0 + i) in ov_off}
                pend.append((t0, nt, wft, S, ov_local))
                if len(pend) > skew:
                    emit_mm2s(pend.popleft())

        while pend:
            emit_mm2s(pend.popleft())
        assert not grp_state

    nc.compile()
    return nc


def _run(inputs, cfg=None, trace=False, tmpdir=None):
    x = np.asarray(inputs["x"], dtype=np.float32)
    w = np.asarray(inputs["w"], dtype=np.float32)
    seg = np.asarray(inputs["seg_i"]).astype(np.int64)
    idx_j = np.asarray(inputs["idx_j"]).astype(np.int64)
    W1 = np.asarray(inputs["W_in2fac"], dtype=np.float32)
    W2 = np.asarray(inputs["W_fac2out"], dtype=np.float32)
    b = np.asarray(inputs["b_fac2out"], dtype=np.float32)

    if cfg is None:
        cfg = Cfg()

    plan = _plan(seg, cfg)

    x16 = x.astype(np.float16)
    w1_16 = np.ascontiguousarray(W1.astype(np.float16))
    w2_16 = np.ascontiguousarray(W2.astype(np.float16))
    bias_16 = np.ascontiguousarray(b[None, :].astype(np.float16))

    in_maps = []
    for c in range(cfg.n_cores):
        in_maps.append(_pack_core(cfg, plan, c, x16, w, idx_j,
                                  w1_16, w2_16, bias_16))

    nc = _build(cfg, plan)

    res = run_bass_kernel_spmd(nc, in_maps, core_ids=list(range(cfg.n_cores)),
                               tmpdir=tmpdir, trace=trace)
    y = np.concatenate(
        [np.asarray(res.results[c]["yT"]).astype(np.float32).T
         for c in range(cfg.n_cores)], axis=0)
    return y[:cfg.na], res, nc, in_maps


def kernel(**inputs) -> np.ndarray:
    y, _res, _nc, _maps = _run(inputs)
    return y
